# revision 45
# baseline (speedup 1.0000x reference)
"""Differential multi-head attention on 8 TRN2 NeuronCores (Bass/Tile).

Problem (hardcoded): B=2, T=N=2048, HID=1024, H=16 heads, DH=64, HALF=32,
DEPTH=6, causal. Reference:
    q = query @ Wq.T ; k = key_ @ Wk.T ; v = value @ Wv.T
    2H half-heads of size 32; att = softmax(causal(q k^T / sqrt(32)))
    att = att_half1 - lambda_full * att_half2        (per head)
    out = att @ v ; RMSNorm(head dim) * g * (1-lambda_init) ; out @ Wo.T

Sharding: batch*heads across 8 cores. Core c handles batch b=c//4 and 4
heads hs=4*(c%4)..hs+4. Host sums the 4 per-core partial Wo projections of
each batch.

v5 schedule ("flipped AV"): one fused stream over 512-query blocks.
  - AV is flipped: stationary = exp-score slice pt[keys, 128q], moving =
    v [keys, 64+1]; output [128 queries, 65] per chunk (~2x fewer AV PE
    cycles vs the [65, queries] orientation; the softmax denominator rides
    along as the ones column of v).
  - PSUM (8 banks): S0/S1 [128,2,512]f32 score ping-pong, WK work tile
    (projection accumulators + y projection), PO for the AV accumulators
    ([s][65*ql..+65] per block). Accumulation groups use one start per
    bank (a start clears the bank's has_written; later chunk groups
    first-write into pending-zero bytes), so 8 chunk-groups can share PO.
  - Per-token stats (queries on partitions): r = lam*kappa*l1/l2
    broadcast along the free dim (stride-0 AP), od = kappa*o1 - r*o2,
    mean-square via DVE free-dim reduce, rsqrt via a DVE Newton
    iteration (bit-trick seed), keeping mid-stream stats off ACT entirely.
    The last sweep's stats read PO directly (skips the o_sB copy).
  - o_norm [tokens, dchan] is DMA-transposed ([128,128] xbar tiles) to
    [dchan, tokens] for the Wo projection; the final 4 y chunks each get
    their own PSUM banks and split per-half writeback DMAs.
  - Projection units / y chunks are PE fillers with per-pp deadlines;
    AV emission trails QK/Exp by 4 steps (in-order PE queue never waits
    on ACT). ACT runs gapless for the middle ~150us; it is the critical
    engine (softmax Exp is ACT-only and ~155us of its ~198us total).
All matmul operands fp16 (fp32 PSUM accumulation); scale 1/sqrt(32) folded
into Wq, g*(1-lambda_init) folded into Wo.
"""

import math
from contextlib import ExitStack

import numpy as np

import concourse.bass as bass
import concourse.tile as tile
from concourse import bacc, bass_isa, mybir
from concourse.bass_utils import run_bass_kernel_spmd

# Prefer the combined ln+exp ACT table set (harmless if only Exp is used).
import concourse.hw_specs as _hw_specs
_orig_get_tables = _hw_specs.get_activation_tables
def _tables_ln_exp_first(arch):
    tabs = _orig_get_tables(arch)
    if "natural_log_exp_and_others" not in tabs:
        return tabs
    return {
        k: (set() if k in ("exp_and_others", "natural_log", "exp_and_friends")
            else v)
        for k, v in tabs.items()
    }
_hw_specs.get_activation_tables = _tables_ln_exp_first
bacc.get_activation_tables = _tables_ln_exp_first

dt = mybir.dt
AF = mybir.ActivationFunctionType
ALU = mybir.AluOpType

B, T, NN, HID = 2, 2048, 2048, 1024
H, DH, HALF = 16, 64, 32
DEPTH = 6
LAMBDA_INIT = 0.8 - 0.6 * math.exp(-0.3 * DEPTH)
EPS = 1e-5
N_CORES = 8
HPC = 4          # heads per core
KAPPA = 0.25     # pre-RMS scale guard (absorbed by the RMS rescale)
LN_BIAS = 1e-10  # rsqrt(0) guard; equivalent eps ~ 1e-5/l1^2 ~ 0

_CACHE = {}


def _build(lam: float, reps: int = 1):
    nc = bacc.Bacc(
        "TRN2", target_bir_lowering=False, debug=False, num_devices=N_CORES
    )

    f16, f32 = dt.float16, dt.float32

    xq_d = nc.dram_tensor("xq", [HID, T], f16, kind="ExternalInput").ap()
    xk_d = nc.dram_tensor("xk", [HID, T], f16, kind="ExternalInput").ap()
    xv_d = nc.dram_tensor("xv", [HID, T], f16, kind="ExternalInput").ap()
    wq_d = nc.dram_tensor("wq", [HID, 256], f16, kind="ExternalInput").ap()
    wk_d = nc.dram_tensor("wk", [HID, 256], f16, kind="ExternalInput").ap()
    wv_d = nc.dram_tensor("wv", [HID, 256], f16, kind="ExternalInput").ap()
    wo_d = nc.dram_tensor("wo", [256, HID], f16, kind="ExternalInput").ap()
    ma_d = nc.dram_tensor("ma", [128, 128], f16, kind="ExternalInput").ap()
    mb_d = nc.dram_tensor("mb", [128, 128], f16, kind="ExternalInput").ap()
    y_d = nc.dram_tensor("y", [T, HID], f16, kind="ExternalOutput").ap()

    lk = lam * KAPPA

    with tile.TileContext(nc) as tc, ExitStack() as ctx:
        ctx.enter_context(
            nc.allow_low_precision(reason="fp16 attention pipeline by design")
        )
        consts = ctx.enter_context(tc.tile_pool(name="consts", bufs=1))
        xpool = ctx.enter_context(tc.tile_pool(name="xpool", bufs=2))
        qkpool = ctx.enter_context(tc.tile_pool(name="qkpool", bufs=1))
        vpool = ctx.enter_context(tc.tile_pool(name="vpool", bufs=1))
        ppool = ctx.enter_context(tc.tile_pool(name="ppool", bufs=4))
        opool = ctx.enter_context(tc.tile_pool(name="opool", bufs=1))
        spool = ctx.enter_context(tc.tile_pool(name="spool", bufs=2))
        ypool = ctx.enter_context(tc.tile_pool(name="ypool", bufs=4))
        psum = ctx.enter_context(tc.tile_pool(name="psum", bufs=1, space="PSUM"))

        for _rep in range(reps):
            # ---------------- persistent tiles ----------------
            wq_s = consts.tile([128, 8, 256], f16, tag="wq")
            wk_s = consts.tile([128, 8, 256], f16, tag="wk")
            wv_s = consts.tile([128, 8, 256], f16, tag="wv")
            wo_s = consts.tile([128, 2, HID], f16, tag="wo")
            ma_s = consts.tile([128, 128], f16, tag="ma")
            mb_s = consts.tile([128, 128], f16, tag="mb")
            ebias = consts.tile([128, 1], f32, tag="ebias")

            qT = [qkpool.tile([128, T], f16, tag=f"qT{pp}", name=f"qT{pp}")
                  for pp in range(2)]
            kT = [qkpool.tile([128, T], f16, tag=f"kT{pp}", name=f"kT{pp}")
                  for pp in range(2)]
            # [keys, nu, pp, g, 64+ones]
            v_s = vpool.tile([128, 16, 2, 2, 65], f16, tag="v", name="v")
            # flipped AV results: [q, qi, g, s, 64+denom] per pp
            o_sB = [opool.tile([128, 16, 2, 2, 65], f16, tag=f"ob{pp}",
                               name=f"ob{pp}") for pp in range(2)]
            # o_norm [tokens, dcol] and transposed [dcol, tokens]
            onB = [opool.tile([128, 16, 128], f16, tag=f"onB{pp}",
                              name=f"onB{pp}") for pp in range(2)]
            onA = [opool.tile([128, 16, 128], f16, tag=f"onA{pp}",
                              name=f"onA{pp}") for pp in range(2)]

            # PSUM: S0+S1 4 banks, WK 2 banks, PO 2 banks
            S2 = [psum.tile([128, 2, 512], f32, tag=f"S{i}", name=f"S{i}")
                  for i in range(2)]
            wk_t = psum.tile([128, 2, 512], f32, tag="WK", name="WK")
            po = psum.tile([128, 2, 512], f32, tag="PO", name="PO")

            def load_xblock(src, c, tag, quarters=False):
                halves = []
                for h in range(2):
                    xt = xpool.tile([128, 4, 512], f16, tag=f"{tag}{h}",
                                    name=f"{tag}{h}_{c}")
                    nq = 2 if quarters else 1
                    for q in range(nq):
                        w = 4 // nq
                        nc.sync.dma_start(
                            out=xt[:, w * q : w * q + w, :],
                            in_=src[512 * h + 128 * w * q :
                                    512 * h + 128 * w * q + 128 * w,
                                    512 * c : 512 * c + 512].rearrange(
                                "(d p) t -> p d t", p=128),
                        )
                    halves.append(xt)
                return lambda d: halves[d // 4][:, d % 4, :]

            # startup: loads ordered for the q->k->first-sweep critical path
            for h in range(2):
                nc.sync.dma_start(
                    out=wq_s[:, 4 * h : 4 * h + 4, :],
                    in_=wq_d[512 * h : 512 * h + 512, :].rearrange(
                        "(d p) j -> p d j", p=128),
                )
            xq0 = load_xblock(xq_d, 0, "xq", quarters=True)
            for h in range(2):
                nc.sync.dma_start(
                    out=wk_s[:, 4 * h : 4 * h + 4, :],
                    in_=wk_d[512 * h : 512 * h + 512, :].rearrange(
                        "(d p) j -> p d j", p=128),
                )
            # block0's xk in COLUMN halves: the first 256 keys land after
            # ~1.5us of DMA, so the first QK steps aren't gated on the
            # full k projection
            xk0c = []
            for ch in range(2):
                xt = xpool.tile([128, 8, 256], f16, tag=f"xkc{ch}",
                                name=f"xkc{ch}")
                nc.sync.dma_start(
                    out=xt,
                    in_=xk_d[:, 256 * ch : 256 * ch + 256].rearrange(
                        "(d p) t -> p d t", p=128),
                )
                xk0c.append(xt)
            nc.sync.dma_start(out=ma_s, in_=ma_d)
            nc.sync.dma_start(out=mb_s, in_=mb_d)
            nc.sync.dma_start(out=wv_s, in_=wv_d.rearrange("(d p) j -> p d j", p=128))
            xv0 = load_xblock(xv_d, 0, "xv")
            nc.sync.dma_start(out=wo_s, in_=wo_d.rearrange("(k p) e -> p k e", p=128))
            nc.gpsimd.memset(v_s[:, :, :, :, 64:65], 1.0)
            nc.gpsimd.memset(ebias, LN_BIAS)

            # ---------------- projection units (PE fillers) ----------------
            wk_bank = [0]  # ping-pong between the two banks of wk_t

            def qk_unit(xd, w_s, dst, pp, c, split_copy=False):
                bk = wk_bank[0]
                wk_bank[0] ^= 1
                acc = wk_t[:, bk, :]
                for d in range(8):
                    nc.tensor.matmul(
                        acc,
                        w_s[:, d, 128 * pp : 128 * pp + 128],
                        xd(d),
                        start=(d == 0),
                        stop=(d == 7),
                    )
                base = 512 * c
                nc.vector.tensor_copy(dst[pp][:, base : base + 512], acc)

            def k_unit_cols(pp):
                # block0 k projection in two 256-key column units, each
                # gated only on its own xk column-half DMA
                for ch in range(2):
                    bk = wk_bank[0]
                    wk_bank[0] ^= 1
                    acc = wk_t[:, bk, 0:256]
                    for d in range(8):
                        nc.tensor.matmul(
                            acc,
                            wk_s[:, d, 128 * pp : 128 * pp + 128],
                            xk0c[ch][:, d, :],
                            start=(d == 0),
                            stop=(d == 7),
                        )
                    nc.vector.tensor_copy(
                        kT[pp][:, 256 * ch : 256 * ch + 256], acc)

            def v_unit(xd, c, nl0):
                for nl in (nl0, nl0 + 1):
                    nu = 4 * c + nl
                    bk = wk_bank[0]
                    wk_bank[0] ^= 1
                    acc = wk_t[:, bk, 0:256]
                    for d in range(8):
                        nc.tensor.matmul(
                            acc,
                            xd(d)[:, 128 * nl : 128 * nl + 128],
                            wv_s[:, d, :],
                            start=(d == 0),
                            stop=(d == 7),
                        )
                    nc.vector.tensor_copy(
                        v_s[:, nu, :, :, 0:64],
                        acc.rearrange("p (w g j) -> p w g j", w=2, g=2),
                    )

            # ---------------- y projection (PE filler) ----------------
            def y_chunk(tt, tail=False, pt_t=None):
                py = wk_t if pt_t is None else pt_t
                for e in range(2):
                    for pp in range(2):
                        nc.tensor.matmul(
                            py[:, e, :],
                            onA[pp][:, tt, :],
                            wo_s[:, pp, 512 * e : 512 * e + 512],
                            start=(pp == 0),
                            stop=(pp == 1),
                            skip_group_check=True,
                        )
                ys = ypool.tile([128, 2, 512], f16, tag="ys", name="ys")
                if tail:
                    # split copies + per-half DMAs so the final writeback
                    # starts as soon as each half lands
                    nc.vector.tensor_copy(ys[:, 0, :], py[:, 0, :])
                    nc.sync.dma_start(
                        out=y_d[128 * tt : 128 * tt + 128, 0:512],
                        in_=ys[:, 0, :],
                    )
                    nc.scalar.copy(ys[:, 1, :], py[:, 1, :])
                    nc.sync.dma_start(
                        out=y_d[128 * tt : 128 * tt + 128, 512:1024],
                        in_=ys[:, 1, :],
                    )
                else:
                    nc.vector.tensor_copy(ys, py)
                    nc.sync.dma_start(
                        out=y_d[128 * tt : 128 * tt + 128, :],
                        in_=ys.rearrange("p e t -> p (e t)"),
                    )

            # ---------------- stats + transpose ----------------
            def stats(pp, c, g, from_po=False, qs=0, qn=4):
                q0 = 4 * c + qs
                if from_po:
                    v4 = po[:, :, 65 * qs : 65 * (qs + qn)].rearrange(
                        "p s (q e) -> p s q e", q=qn)
                    l1 = v4[:, 0, :, 64]
                    l2 = v4[:, 1, :, 64]
                    o1 = v4[:, 0, :, 0:64]
                    o2 = v4[:, 1, :, 0:64]
                else:
                    sl = o_sB[pp]
                    l1 = sl[:, q0 : q0 + qn, g, 0, 64]
                    l2 = sl[:, q0 : q0 + qn, g, 1, 64]
                    o1 = sl[:, q0 : q0 + qn, g, 0, 0:64]
                    o2 = sl[:, q0 : q0 + qn, g, 1, 0:64]
                r2 = spool.tile([128, 4], f16, tag="r2", name="r2")[:, 0:qn]
                nc.vector.reciprocal(r2, l2)
                rr = spool.tile([128, 4], f16, tag="rr", name="rr")[:, 0:qn]
                nc.vector.scalar_tensor_tensor(
                    out=rr, in0=l1, scalar=lk, in1=r2,
                    op0=ALU.mult, op1=ALU.mult,
                )
                rb = rr.unsqueeze(2).broadcast_to([128, qn, 64])
                m2 = spool.tile([128, 4, 64], f16, tag="m2",
                                name="m2")[:, 0:qn, :]
                nc.vector.tensor_mul(m2, o2, rb)
                od = spool.tile([128, 4, 64], f16, tag="od", name="od",
                                bufs=2)[:, 0:qn, :]
                nc.vector.scalar_tensor_tensor(
                    out=od, in0=o1, scalar=KAPPA, in1=m2,
                    op0=ALU.mult, op1=ALU.subtract,
                )
                sq = spool.tile([128, 4, 64], f32, tag="sq",
                                name="sq")[:, 0:qn, :]
                nc.vector.tensor_mul(sq, od, od)
                ms = spool.tile([128, 4], f32, tag="ms", name="ms")[:, 0:qn]
                nc.vector.tensor_reduce(ms, sq, mybir.AxisListType.X, ALU.add)
                sr = spool.tile([128, 4], f16, tag="sr", name="sr")[:, 0:qn]
                if from_po:
                    # tail: ACT is idle here, keep the short Ln/Exp chain
                    nc.scalar.activation(out=sr, in_=ms, func=AF.Ln,
                                         scale=1.0 / DH, bias=ebias)
                    nc.scalar.activation(out=sr, in_=sr, func=AF.Exp,
                                         scale=-0.5)
                else:
                    # rsqrt on DVE (bit-trick seed + 2 Newton steps) keeps
                    # the mid-stream off the saturated ACT engine
                    msb = spool.tile([128, 4], f32, tag="msb",
                                     name="msb")[:, 0:qn]
                    nc.vector.tensor_scalar(msb, ms, 1.0 / DH, LN_BIAS,
                                            ALU.mult, ALU.add)
                    shv = spool.tile([128, 4], dt.int32, tag="shv",
                                     name="shv")[:, 0:qn]
                    nc.vector.tensor_scalar(shv, msb.bitcast(dt.int32), 1,
                                            None, ALU.arith_shift_right)
                    y0 = spool.tile([128, 4], f32, tag="y0",
                                    name="y0")[:, 0:qn]
                    nc.vector.tensor_scalar(y0.bitcast(dt.int32), shv, -1,
                                            0x5F3759DF, ALU.mult, ALU.add)
                    yy = y0
                    for _it in range(2):
                        t1 = spool.tile([128, 4], f32, tag="t1",
                                        name="t1")[:, 0:qn]
                        nc.vector.tensor_mul(t1, yy, yy)
                        t2 = spool.tile([128, 4], f32, tag="t2",
                                        name="t2")[:, 0:qn]
                        nc.vector.tensor_mul(t2, t1, msb)
                        t3 = spool.tile([128, 4], f32, tag="t3",
                                        name="t3")[:, 0:qn]
                        nc.vector.tensor_scalar(t3, t2, -0.5, 1.5,
                                                ALU.mult, ALU.add)
                        yn = (spool.tile([128, 4], f32, tag="yn",
                                         name="yn")[:, 0:qn]
                              if _it == 0 else sr)
                        nc.vector.tensor_mul(yn, yy, t3)
                        yy = yn
                srb = sr.unsqueeze(2).broadcast_to([128, qn, 64])
                nc.vector.tensor_mul(
                    onB[pp][:, q0 : q0 + qn, 64 * g : 64 * g + 64], od, srb
                )

            def transpose_block(pp, c):
                for ql in range(4):
                    tt = 4 * c + ql
                    nc.sync.dma_start_transpose(
                        out=onA[pp][:, tt, :], in_=onB[pp][:, tt, :]
                    )

            # ---------------- attention sweep ----------------
            pend = []   # deferred closures (AV matmuls etc.)

            def drain(to):
                while len(pend) > to:
                    pend.pop(0)()

            fillers = []  # PE filler closures (proj units, y chunks)

            def pop_filler():
                if fillers:
                    fillers.pop(0)()

            def av_closure(nu, j, pts, pp, g, last):
                def av_op():
                    for s in range(2):
                        for ql in range(max(j, 0), 4):
                            first = (nu == 0 and ql == 0)
                            nc.tensor.matmul(
                                po[:, s, 65 * ql : 65 * ql + 65],
                                pts[:, s, 128 * ql : 128 * ql + 128],
                                v_s[:, nu, pp, g, :],
                                start=first,
                                stop=(nu == last and ql == 3),
                                skip_group_check=True,
                            )
                return av_op

            def sweep(pp, c, g, fill_every, pair=False):
                last = 4 * c + 3
                step = 0
                for nu in range(last + 1):
                    j = nu - 4 * c
                    diag = j >= 0
                    lo = 128 * j if diag else 0
                    half = nu % 2
                    S = S2[half]
                    for s in range(2):
                        hh = 2 * g + s
                        kk = kT[pp][32 * hh : 32 * hh + 32,
                                    128 * nu : 128 * nu + 128]
                        qq = qT[pp][32 * hh : 32 * hh + 32,
                                    512 * c + lo : 512 * c + 512]
                        if diag:
                            nc.tensor.matmul(
                                S[:, s, lo:512], kk, qq,
                                start=True, stop=False,
                                tile_position=(32 * hh, 0),
                            )
                            nc.tensor.matmul(
                                S[:, s, lo : lo + 128], ma_s, mb_s,
                                start=False, stop=True,
                                tile_position=(0, 0),
                            )
                        else:
                            nc.tensor.matmul(
                                S[:, s, :], kk, qq,
                                start=True, stop=True,
                                tile_position=(32 * hh, 0),
                            )
                    if pair and not diag:
                        if half == 0:
                            # defer: the pair's Exp fires on the odd step
                            step += 1
                            if fill_every and step % fill_every == 0:
                                pop_filler()
                            continue
                        raise NotImplementedError("exp pairing disabled")
                    else:
                        pt = ppool.tile([128, 2, 512], f16, tag="pt",
                                        name="pt", bufs=7)
                        nc.scalar.activation(
                            out=pt[:, :, lo:512], in_=S[:, :, lo:512],
                            func=AF.Exp,
                        )
                        pend.append(av_closure(nu, j, pt, pp, g, last))
                    drain(2)
                    step += 1
                    if fill_every and step % fill_every == 0:
                        pop_filler()

                last_sweep = (pp == 1 and c == 3 and g == 1)

                def end_ops(pp=pp, c=c, g=g, last_sweep=last_sweep):
                    if last_sweep:
                        # split into two half-chains so the first transposes
                        # (and tail y chunks) start earlier
                        for qs in (0, 2):
                            stats(pp, c, g, from_po=True, qs=qs, qn=2)
                            for ql in (qs, qs + 1):
                                tt = 4 * c + ql
                                nc.sync.dma_start_transpose(
                                    out=onA[pp][:, tt, :],
                                    in_=onB[pp][:, tt, :],
                                )
                        return
                    # po -> o_sB (frees PO for the next sweep)
                    nc.vector.tensor_copy(
                        o_sB[pp][:, 4 * c : 4 * c + 4, g, :, :].transpose(
                            [0, 2, 1, 3]),
                        po[:, :, 0:260].rearrange(
                            "p s (q e) -> p s q e", q=4),
                    )
                    stats(pp, c, g)
                    if g == 1:
                        transpose_block(pp, c)

                pend.append(end_ops)

            # ---------------- the stream ----------------
            xs = {0: (xq0, None, xv0)}

            def prefetch(c):
                if c <= 3 and c not in xs:
                    xs[c] = (load_xblock(xq_d, c, "xq"),
                             load_xblock(xk_d, c, "xk"),
                             load_xblock(xv_d, c, "xv"))

            # block 0 critical path: q then k for pp0, then sweeps start
            xq_c, xk_c, xv_c = xs[0]
            qk_unit(xq_c, wq_s, qT, 0, 0)
            k_unit_cols(0)
            prefetch(1)
            xq_1, xk_1, _ = xs[1]
            fillers.extend([
                lambda: v_unit(xv_c, 0, 0),
                lambda: v_unit(xv_c, 0, 2),
                lambda: qk_unit(xq_c, wq_s, qT, 1, 0),
                lambda: k_unit_cols(1),
            ])
            late0 = [
                lambda: qk_unit(xq_1, wq_s, qT, 0, 1),
                lambda: qk_unit(xk_1, wk_s, kT, 0, 1),
            ]

            def block_fillers(c):
                # units needed during block c's sweeps: v(c) (AV nu>=4c),
                # qk(c,pp1) (before block c's pp1 sweeps), qk(c+1,pp0)
                # (before block c+1's first sweep)
                xq_c, xk_c, xv_c = xs[c]
                out = [
                    lambda: v_unit(xv_c, c, 0),
                    lambda: v_unit(xv_c, c, 2),
                    lambda: qk_unit(xq_c, wq_s, qT, 1, c),
                    lambda: qk_unit(xk_c, wk_s, kT, 1, c),
                ]
                if c + 1 <= 3:
                    xq_n, xk_n, xv_n = xs[c + 1]
                    out += [
                        lambda: qk_unit(xq_n, wq_s, qT, 0, c + 1),
                        lambda: qk_unit(xk_n, wk_s, kT, 0, c + 1),
                    ]
                return out

            FILL_EVERY = {0: 2, 1: 4, 2: 4, 3: 5}
            for c in range(4):
                prefetch(c + 1)
                if c == 0:
                    # qk(1,pp0) drips in during block0's pp1 sweeps
                    fillers.extend(late0)
                if c >= 1:
                    fillers.extend(block_fillers(c))
                if c == 2:
                    fillers.extend(
                        [lambda tt=tt: y_chunk(tt) for tt in range(0, 4)])
                if c == 3:
                    fillers.extend(
                        [lambda tt=tt: y_chunk(tt) for tt in range(4, 12)])
                for pp in range(2):
                    for g in range(2):
                        sweep(pp, c, g, FILL_EVERY[c])
                # anything not yet emitted (proj must precede block c+1)
                while fillers:
                    pop_filler()

            drain(0)
            # tail: each chunk on its own psum banks, no bank-reuse waits
            for i, tt in enumerate(range(12, 16)):
                y_chunk(tt, tail=True,
                        pt_t=[S2[0], S2[1], wk_t, po][i])

    nc.compile()
    return nc


def _prep(inputs):
    a = {k: np.asarray(v) for k, v in inputs.items()}
    lam = float(
        np.exp(np.sum(a["lq1"] * a["lk1"], dtype=np.float32))
        - np.exp(np.sum(a["lq2"] * a["lk2"], dtype=np.float32))
        + LAMBDA_INIT
    )
    wq_t = (a["Wq"].T / math.sqrt(HALF)).astype(np.float16)
    wk_t = a["Wk"].T.astype(np.float16)
    wv_t = a["Wv"].T.astype(np.float16)
    wo_g = (a["Wo"] * (np.tile(a["g"], H) * (1.0 - LAMBDA_INIT))[None, :]).T.astype(
        np.float16
    )
    r = np.arange(128)
    # ramp mask pair: (ma^T mb)[n, t] = -64*(n - t) for n > t else 0
    ma = (8.0 * (r[:, None] < r[None, :])).astype(np.float16)       # [d, n]
    mb = (-8.0 * (r[:, None] >= r[None, :])).astype(np.float16)     # [d, t]

    in_maps = []
    for core in range(N_CORES):
        b, hs = core // 4, 4 * (core % 4)
        sl = slice(DH * hs, DH * hs + DH * HPC)
        in_maps.append({
            "xq": np.ascontiguousarray(a["query"][b].T).astype(np.float16),
            "xk": np.ascontiguousarray(a["key_"][b].T).astype(np.float16),
            "xv": np.ascontiguousarray(a["value"][b].T).astype(np.float16),
            "wq": np.ascontiguousarray(wq_t[:, sl]),
            "wk": np.ascontiguousarray(wk_t[:, sl]),
            "wv": np.ascontiguousarray(wv_t[:, sl]),
            "wo": np.ascontiguousarray(wo_g[sl, :]),
            "ma": ma,
            "mb": mb,
        })
    return lam, in_maps


def run(inputs, trace=False, reps=1):
    lam, in_maps = _prep(inputs)
    key = (round(lam, 6), reps)
    if key not in _CACHE:
        _CACHE[key] = _build(lam, reps)
    nc = _CACHE[key]
    res = run_bass_kernel_spmd(
        nc, in_maps, core_ids=list(range(N_CORES)), trace=trace
    )
    out = np.empty((B, T, HID), np.float32)
    for b in range(B):
        out[b] = sum(res.results[4 * b + i]["y"].astype(np.float32) for i in range(4))
    return out, res


def kernel(**inputs) -> np.ndarray:
    out, _ = run(inputs)
    return out


# revision 49
# speedup vs baseline: 1.0022x; 1.0022x over previous
"""Differential multi-head attention on 8 TRN2 NeuronCores (Bass/Tile).

Problem (hardcoded): B=2, T=N=2048, HID=1024, H=16 heads, DH=64, HALF=32,
DEPTH=6, causal. Reference:
    q = query @ Wq.T ; k = key_ @ Wk.T ; v = value @ Wv.T
    2H half-heads of size 32; att = softmax(causal(q k^T / sqrt(32)))
    att = att_half1 - lambda_full * att_half2        (per head)
    out = att @ v ; RMSNorm(head dim) * g * (1-lambda_init) ; out @ Wo.T

Sharding: batch*heads across 8 cores. Core c handles batch b=c//4 and 4
heads hs=4*(c%4)..hs+4. Host sums the 4 per-core partial Wo projections of
each batch.

v5 schedule ("flipped AV"): one fused stream over 512-query blocks.
  - AV is flipped: stationary = exp-score slice pt[keys, 128q], moving =
    v [keys, 64+1]; output [128 queries, 65] per chunk (~2x fewer AV PE
    cycles vs the [65, queries] orientation; the softmax denominator rides
    along as the ones column of v).
  - PSUM (8 banks): S0/S1 [128,2,512]f32 score ping-pong, WK work tile
    (projection accumulators + y projection), PO for the AV accumulators
    ([s][65*ql..+65] per block). Accumulation groups use one start per
    bank (a start clears the bank's has_written; later chunk groups
    first-write into pending-zero bytes), so 8 chunk-groups can share PO.
  - Per-token stats (queries on partitions): r = lam*kappa*l1/l2
    broadcast along the free dim (stride-0 AP), od = kappa*o1 - r*o2,
    mean-square via DVE free-dim reduce, rsqrt via a DVE Newton
    iteration (bit-trick seed), keeping mid-stream stats off ACT entirely.
    The last sweep's stats read PO directly (skips the o_sB copy).
  - o_norm [tokens, dchan] is DMA-transposed ([128,128] xbar tiles) to
    [dchan, tokens] for the Wo projection; the final 4 y chunks each get
    their own PSUM banks and split per-half writeback DMAs.
  - Projection units / y chunks are PE fillers with per-pp deadlines;
    AV emission trails QK/Exp by 4 steps (in-order PE queue never waits
    on ACT). ACT runs gapless for the middle ~150us; it is the critical
    engine (softmax Exp is ACT-only and ~155us of its ~198us total).
All matmul operands fp16 (fp32 PSUM accumulation); scale 1/sqrt(32) folded
into Wq, g*(1-lambda_init) folded into Wo.
"""

import math
from contextlib import ExitStack

import numpy as np

import concourse.bass as bass
import concourse.tile as tile
from concourse import bacc, bass_isa, mybir
from concourse.bass_utils import run_bass_kernel_spmd

# Prefer the combined ln+exp ACT table set (harmless if only Exp is used).
import concourse.hw_specs as _hw_specs
_orig_get_tables = _hw_specs.get_activation_tables
def _tables_ln_exp_first(arch):
    tabs = _orig_get_tables(arch)
    if "natural_log_exp_and_others" not in tabs:
        return tabs
    return {
        k: (set() if k in ("exp_and_others", "natural_log", "exp_and_friends")
            else v)
        for k, v in tabs.items()
    }
_hw_specs.get_activation_tables = _tables_ln_exp_first
bacc.get_activation_tables = _tables_ln_exp_first

dt = mybir.dt
AF = mybir.ActivationFunctionType
ALU = mybir.AluOpType

B, T, NN, HID = 2, 2048, 2048, 1024
H, DH, HALF = 16, 64, 32
DEPTH = 6
LAMBDA_INIT = 0.8 - 0.6 * math.exp(-0.3 * DEPTH)
EPS = 1e-5
N_CORES = 8
HPC = 4          # heads per core
KAPPA = 0.25     # pre-RMS scale guard (absorbed by the RMS rescale)
LN_BIAS = 1e-10  # rsqrt(0) guard; equivalent eps ~ 1e-5/l1^2 ~ 0

_CACHE = {}


def _build(lam: float, reps: int = 1):
    nc = bacc.Bacc(
        "TRN2", target_bir_lowering=False, debug=False, num_devices=N_CORES
    )

    f16, f32 = dt.float16, dt.float32

    xq_d = nc.dram_tensor("xq", [HID, T], f16, kind="ExternalInput").ap()
    xk_d = nc.dram_tensor("xk", [HID, T], f16, kind="ExternalInput").ap()
    xv_d = nc.dram_tensor("xv", [HID, T], f16, kind="ExternalInput").ap()
    wq_d = nc.dram_tensor("wq", [HID, 256], f16, kind="ExternalInput").ap()
    wk_d = nc.dram_tensor("wk", [HID, 256], f16, kind="ExternalInput").ap()
    wv_d = nc.dram_tensor("wv", [HID, 256], f16, kind="ExternalInput").ap()
    wo_d = nc.dram_tensor("wo", [256, HID], f16, kind="ExternalInput").ap()
    ma_d = nc.dram_tensor("ma", [128, 128], f16, kind="ExternalInput").ap()
    mb_d = nc.dram_tensor("mb", [128, 128], f16, kind="ExternalInput").ap()
    y_d = nc.dram_tensor("y", [T, HID], f16, kind="ExternalOutput").ap()

    lk = lam * KAPPA

    with tile.TileContext(nc) as tc, ExitStack() as ctx:
        ctx.enter_context(
            nc.allow_low_precision(reason="fp16 attention pipeline by design")
        )
        consts = ctx.enter_context(tc.tile_pool(name="consts", bufs=1))
        xpool = ctx.enter_context(tc.tile_pool(name="xpool", bufs=2))
        qkpool = ctx.enter_context(tc.tile_pool(name="qkpool", bufs=1))
        vpool = ctx.enter_context(tc.tile_pool(name="vpool", bufs=1))
        ppool = ctx.enter_context(tc.tile_pool(name="ppool", bufs=4))
        opool = ctx.enter_context(tc.tile_pool(name="opool", bufs=1))
        spool = ctx.enter_context(tc.tile_pool(name="spool", bufs=2))
        ypool = ctx.enter_context(tc.tile_pool(name="ypool", bufs=4))
        psum = ctx.enter_context(tc.tile_pool(name="psum", bufs=1, space="PSUM"))

        for _rep in range(reps):
            # ---------------- persistent tiles ----------------
            wq_s = consts.tile([128, 8, 256], f16, tag="wq")
            wk_s = consts.tile([128, 8, 256], f16, tag="wk")
            wv_s = consts.tile([128, 8, 256], f16, tag="wv")
            wo_s = consts.tile([128, 2, HID], f16, tag="wo")
            ma_s = consts.tile([128, 128], f16, tag="ma")
            mb_s = consts.tile([128, 128], f16, tag="mb")
            ebias = consts.tile([128, 1], f32, tag="ebias")

            qT = [qkpool.tile([128, T], f16, tag=f"qT{pp}", name=f"qT{pp}")
                  for pp in range(2)]
            kT = [qkpool.tile([128, T], f16, tag=f"kT{pp}", name=f"kT{pp}")
                  for pp in range(2)]
            # [keys, nu, pp, g, 64+ones]
            v_s = vpool.tile([128, 16, 2, 2, 65], f16, tag="v", name="v")
            # flipped AV results: [q, qi, g, s, 64+denom] per pp
            o_sB = [opool.tile([128, 16, 2, 2, 65], f16, tag=f"ob{pp}",
                               name=f"ob{pp}") for pp in range(2)]
            # o_norm [tokens, dcol] and transposed [dcol, tokens]
            onB = [opool.tile([128, 16, 128], f16, tag=f"onB{pp}",
                              name=f"onB{pp}") for pp in range(2)]
            onA = [opool.tile([128, 16, 128], f16, tag=f"onA{pp}",
                              name=f"onA{pp}") for pp in range(2)]

            # PSUM: S0+S1 4 banks, WK 2 banks, PO 2 banks
            S2 = [psum.tile([128, 2, 512], f32, tag=f"S{i}", name=f"S{i}")
                  for i in range(2)]
            wk_t = psum.tile([128, 2, 512], f32, tag="WK", name="WK")
            po = psum.tile([128, 2, 512], f32, tag="PO", name="PO")

            def load_xblock(src, c, tag, quarters=False):
                halves = []
                for h in range(2):
                    xt = xpool.tile([128, 4, 512], f16, tag=f"{tag}{h}",
                                    name=f"{tag}{h}_{c}")
                    nq = 2 if quarters else 1
                    for q in range(nq):
                        w = 4 // nq
                        nc.sync.dma_start(
                            out=xt[:, w * q : w * q + w, :],
                            in_=src[512 * h + 128 * w * q :
                                    512 * h + 128 * w * q + 128 * w,
                                    512 * c : 512 * c + 512].rearrange(
                                "(d p) t -> p d t", p=128),
                        )
                    halves.append(xt)
                return lambda d: halves[d // 4][:, d % 4, :]

            # startup: loads ordered for the q->k->first-sweep critical path
            for h in range(2):
                nc.sync.dma_start(
                    out=wq_s[:, 4 * h : 4 * h + 4, :],
                    in_=wq_d[512 * h : 512 * h + 512, :].rearrange(
                        "(d p) j -> p d j", p=128),
                )
            xq0 = load_xblock(xq_d, 0, "xq", quarters=True)
            for h in range(2):
                nc.sync.dma_start(
                    out=wk_s[:, 4 * h : 4 * h + 4, :],
                    in_=wk_d[512 * h : 512 * h + 512, :].rearrange(
                        "(d p) j -> p d j", p=128),
                )
            xk0 = load_xblock(xk_d, 0, "xk", quarters=True)
            nc.sync.dma_start(out=ma_s, in_=ma_d)
            nc.sync.dma_start(out=mb_s, in_=mb_d)
            nc.sync.dma_start(out=wv_s, in_=wv_d.rearrange("(d p) j -> p d j", p=128))
            xv0 = load_xblock(xv_d, 0, "xv")
            nc.sync.dma_start(out=wo_s, in_=wo_d.rearrange("(k p) e -> p k e", p=128))
            nc.gpsimd.memset(v_s[:, :, :, :, 64:65], 1.0)
            nc.gpsimd.memset(ebias, LN_BIAS)

            # ---------------- projection units (PE fillers) ----------------
            wk_bank = [0]  # ping-pong between the two banks of wk_t

            def qk_unit(xd, w_s, dst, pp, c, split_copy=False):
                bk = wk_bank[0]
                wk_bank[0] ^= 1
                acc = wk_t[:, bk, :]
                for d in range(8):
                    nc.tensor.matmul(
                        acc,
                        w_s[:, d, 128 * pp : 128 * pp + 128],
                        xd(d),
                        start=(d == 0),
                        stop=(d == 7),
                    )
                base = 512 * c
                nc.vector.tensor_copy(dst[pp][:, base : base + 512], acc)

            def v_unit(xd, c, nl0):
                for nl in (nl0, nl0 + 1):
                    nu = 4 * c + nl
                    bk = wk_bank[0]
                    wk_bank[0] ^= 1
                    acc = wk_t[:, bk, 0:256]
                    for d in range(8):
                        nc.tensor.matmul(
                            acc,
                            xd(d)[:, 128 * nl : 128 * nl + 128],
                            wv_s[:, d, :],
                            start=(d == 0),
                            stop=(d == 7),
                        )
                    nc.vector.tensor_copy(
                        v_s[:, nu, :, :, 0:64],
                        acc.rearrange("p (w g j) -> p w g j", w=2, g=2),
                    )

            # ---------------- y projection (PE filler) ----------------
            def y_chunk(tt, tail=False, pt_t=None):
                py = wk_t if pt_t is None else pt_t
                for e in range(2):
                    for pp in range(2):
                        nc.tensor.matmul(
                            py[:, e, :],
                            onA[pp][:, tt, :],
                            wo_s[:, pp, 512 * e : 512 * e + 512],
                            start=(pp == 0),
                            stop=(pp == 1),
                            skip_group_check=True,
                        )
                ys = ypool.tile([128, 2, 512], f16, tag="ys", name="ys")
                if tail:
                    # split copies + per-half DMAs so the final writeback
                    # starts as soon as each half lands
                    nc.vector.tensor_copy(ys[:, 0, :], py[:, 0, :])
                    nc.sync.dma_start(
                        out=y_d[128 * tt : 128 * tt + 128, 0:512],
                        in_=ys[:, 0, :],
                    )
                    nc.scalar.copy(ys[:, 1, :], py[:, 1, :])
                    nc.sync.dma_start(
                        out=y_d[128 * tt : 128 * tt + 128, 512:1024],
                        in_=ys[:, 1, :],
                    )
                else:
                    nc.vector.tensor_copy(ys, py)
                    nc.sync.dma_start(
                        out=y_d[128 * tt : 128 * tt + 128, :],
                        in_=ys.rearrange("p e t -> p (e t)"),
                    )

            # ---------------- stats + transpose ----------------
            def stats(pp, c, g, from_po=False, qs=0, qn=4):
                q0 = 4 * c + qs
                if from_po:
                    v4 = po[:, :, 65 * qs : 65 * (qs + qn)].rearrange(
                        "p s (q e) -> p s q e", q=qn)
                    l1 = v4[:, 0, :, 64]
                    l2 = v4[:, 1, :, 64]
                    o1 = v4[:, 0, :, 0:64]
                    o2 = v4[:, 1, :, 0:64]
                else:
                    sl = o_sB[pp]
                    l1 = sl[:, q0 : q0 + qn, g, 0, 64]
                    l2 = sl[:, q0 : q0 + qn, g, 1, 64]
                    o1 = sl[:, q0 : q0 + qn, g, 0, 0:64]
                    o2 = sl[:, q0 : q0 + qn, g, 1, 0:64]
                r2 = spool.tile([128, 4], f16, tag="r2", name="r2")[:, 0:qn]
                nc.vector.reciprocal(r2, l2)
                rr = spool.tile([128, 4], f16, tag="rr", name="rr")[:, 0:qn]
                nc.vector.scalar_tensor_tensor(
                    out=rr, in0=l1, scalar=lk, in1=r2,
                    op0=ALU.mult, op1=ALU.mult,
                )
                rb = rr.unsqueeze(2).broadcast_to([128, qn, 64])
                m2 = spool.tile([128, 4, 64], f16, tag="m2",
                                name="m2")[:, 0:qn, :]
                nc.vector.tensor_mul(m2, o2, rb)
                od = spool.tile([128, 4, 64], f16, tag="od", name="od",
                                bufs=2)[:, 0:qn, :]
                nc.vector.scalar_tensor_tensor(
                    out=od, in0=o1, scalar=KAPPA, in1=m2,
                    op0=ALU.mult, op1=ALU.subtract,
                )
                sq = spool.tile([128, 4, 64], f32, tag="sq",
                                name="sq")[:, 0:qn, :]
                nc.vector.tensor_mul(sq, od, od)
                ms = spool.tile([128, 4], f32, tag="ms", name="ms")[:, 0:qn]
                nc.vector.tensor_reduce(ms, sq, mybir.AxisListType.X, ALU.add)
                sr = spool.tile([128, 4], f16, tag="sr", name="sr")[:, 0:qn]
                if from_po:
                    # tail: ACT is idle here, keep the short Ln/Exp chain
                    nc.scalar.activation(out=sr, in_=ms, func=AF.Ln,
                                         scale=1.0 / DH, bias=ebias)
                    nc.scalar.activation(out=sr, in_=sr, func=AF.Exp,
                                         scale=-0.5)
                else:
                    # rsqrt on DVE (bit-trick seed + 2 Newton steps) keeps
                    # the mid-stream off the saturated ACT engine
                    msb = spool.tile([128, 4], f32, tag="msb",
                                     name="msb")[:, 0:qn]
                    nc.vector.tensor_scalar(msb, ms, 1.0 / DH, LN_BIAS,
                                            ALU.mult, ALU.add)
                    shv = spool.tile([128, 4], dt.int32, tag="shv",
                                     name="shv")[:, 0:qn]
                    nc.vector.tensor_scalar(shv, msb.bitcast(dt.int32), 1,
                                            None, ALU.arith_shift_right)
                    y0 = spool.tile([128, 4], f32, tag="y0",
                                    name="y0")[:, 0:qn]
                    nc.vector.tensor_scalar(y0.bitcast(dt.int32), shv, -1,
                                            0x5F3759DF, ALU.mult, ALU.add)
                    yy = y0
                    for _it in range(2):
                        t1 = spool.tile([128, 4], f32, tag="t1",
                                        name="t1")[:, 0:qn]
                        nc.vector.tensor_mul(t1, yy, yy)
                        t2 = spool.tile([128, 4], f32, tag="t2",
                                        name="t2")[:, 0:qn]
                        nc.vector.tensor_mul(t2, t1, msb)
                        t3 = spool.tile([128, 4], f32, tag="t3",
                                        name="t3")[:, 0:qn]
                        nc.vector.tensor_scalar(t3, t2, -0.5, 1.5,
                                                ALU.mult, ALU.add)
                        yn = (spool.tile([128, 4], f32, tag="yn",
                                         name="yn")[:, 0:qn]
                              if _it == 0 else sr)
                        nc.vector.tensor_mul(yn, yy, t3)
                        yy = yn
                srb = sr.unsqueeze(2).broadcast_to([128, qn, 64])
                nc.vector.tensor_mul(
                    onB[pp][:, q0 : q0 + qn, 64 * g : 64 * g + 64], od, srb
                )

            def transpose_block(pp, c):
                for ql in range(4):
                    tt = 4 * c + ql
                    nc.sync.dma_start_transpose(
                        out=onA[pp][:, tt, :], in_=onB[pp][:, tt, :]
                    )

            # ---------------- attention sweep ----------------
            pend = []   # deferred closures (AV matmuls etc.)

            def drain(to):
                while len(pend) > to:
                    pend.pop(0)()

            fillers = []  # PE filler closures (proj units, y chunks)

            def pop_filler():
                if fillers:
                    fillers.pop(0)()

            def av_closure(nu, j, pts, pp, g, last):
                def av_op():
                    for s in range(2):
                        for ql in range(max(j, 0), 4):
                            first = (nu == 0 and ql == 0)
                            nc.tensor.matmul(
                                po[:, s, 65 * ql : 65 * ql + 65],
                                pts[:, s, 128 * ql : 128 * ql + 128],
                                v_s[:, nu, pp, g, :],
                                start=first,
                                stop=(nu == last and ql == 3),
                                skip_group_check=True,
                            )
                return av_op

            def sweep(pp, c, g, fill_every, pair=False):
                last = 4 * c + 3
                step = 0
                for nu in range(last + 1):
                    j = nu - 4 * c
                    diag = j >= 0
                    lo = 128 * j if diag else 0
                    half = nu % 2
                    S = S2[half]
                    for s in range(2):
                        hh = 2 * g + s
                        kk = kT[pp][32 * hh : 32 * hh + 32,
                                    128 * nu : 128 * nu + 128]
                        qq = qT[pp][32 * hh : 32 * hh + 32,
                                    512 * c + lo : 512 * c + 512]
                        if diag:
                            nc.tensor.matmul(
                                S[:, s, lo:512], kk, qq,
                                start=True, stop=False,
                                tile_position=(32 * hh, 0),
                            )
                            nc.tensor.matmul(
                                S[:, s, lo : lo + 128], ma_s, mb_s,
                                start=False, stop=True,
                                tile_position=(0, 0),
                            )
                        else:
                            nc.tensor.matmul(
                                S[:, s, :], kk, qq,
                                start=True, stop=True,
                                tile_position=(32 * hh, 0),
                            )
                    if pair and not diag:
                        if half == 0:
                            # defer: the pair's Exp fires on the odd step
                            step += 1
                            if fill_every and step % fill_every == 0:
                                pop_filler()
                            continue
                        raise NotImplementedError("exp pairing disabled")
                    else:
                        pt = ppool.tile([128, 2, 512], f16, tag="pt",
                                        name="pt", bufs=7)
                        nc.scalar.activation(
                            out=pt[:, :, lo:512], in_=S[:, :, lo:512],
                            func=AF.Exp,
                        )
                        pend.append(av_closure(nu, j, pt, pp, g, last))
                    drain(2)
                    step += 1
                    if fill_every and step % fill_every == 0:
                        pop_filler()

                last_sweep = (pp == 1 and c == 3 and g == 1)

                def end_ops(pp=pp, c=c, g=g, last_sweep=last_sweep):
                    if last_sweep:
                        # split into two half-chains so the first transposes
                        # (and tail y chunks) start earlier
                        for qs in (0, 2):
                            stats(pp, c, g, from_po=True, qs=qs, qn=2)
                            for ql in (qs, qs + 1):
                                tt = 4 * c + ql
                                nc.sync.dma_start_transpose(
                                    out=onA[pp][:, tt, :],
                                    in_=onB[pp][:, tt, :],
                                )
                        return
                    # po -> o_sB (frees PO for the next sweep)
                    nc.vector.tensor_copy(
                        o_sB[pp][:, 4 * c : 4 * c + 4, g, :, :].transpose(
                            [0, 2, 1, 3]),
                        po[:, :, 0:260].rearrange(
                            "p s (q e) -> p s q e", q=4),
                    )
                    stats(pp, c, g)
                    if g == 1:
                        transpose_block(pp, c)

                pend.append(end_ops)

            # ---------------- the stream ----------------
            xs = {0: (xq0, xk0, xv0)}

            def prefetch(c):
                if c <= 3 and c not in xs:
                    xs[c] = (load_xblock(xq_d, c, "xq"),
                             load_xblock(xk_d, c, "xk"),
                             load_xblock(xv_d, c, "xv"))

            # block 0 critical path: q then k for pp0, then sweeps start
            xq_c, xk_c, xv_c = xs[0]
            qk_unit(xq_c, wq_s, qT, 0, 0)
            qk_unit(xk_c, wk_s, kT, 0, 0)
            prefetch(1)
            xq_1, xk_1, _ = xs[1]
            fillers.extend([
                lambda: v_unit(xv_c, 0, 0),
                lambda: v_unit(xv_c, 0, 2),
                lambda: qk_unit(xq_c, wq_s, qT, 1, 0),
                lambda: qk_unit(xk_c, wk_s, kT, 1, 0),
            ])
            late0 = [
                lambda: qk_unit(xq_1, wq_s, qT, 0, 1),
                lambda: qk_unit(xk_1, wk_s, kT, 0, 1),
            ]

            def block_fillers(c):
                # units needed during block c's sweeps: v(c) (AV nu>=4c),
                # qk(c,pp1) (before block c's pp1 sweeps), qk(c+1,pp0)
                # (before block c+1's first sweep)
                xq_c, xk_c, xv_c = xs[c]
                out = [
                    lambda: v_unit(xv_c, c, 0),
                    lambda: v_unit(xv_c, c, 2),
                    lambda: qk_unit(xq_c, wq_s, qT, 1, c),
                    lambda: qk_unit(xk_c, wk_s, kT, 1, c),
                ]
                if c + 1 <= 3:
                    xq_n, xk_n, xv_n = xs[c + 1]
                    out += [
                        lambda: qk_unit(xq_n, wq_s, qT, 0, c + 1),
                        lambda: qk_unit(xk_n, wk_s, kT, 0, c + 1),
                    ]
                return out

            FILL_EVERY = {0: 2, 1: 4, 2: 4, 3: 5}
            for c in range(4):
                prefetch(c + 1)
                if c == 0:
                    # qk(1,pp0) drips in during block0's pp1 sweeps
                    fillers.extend(late0)
                if c >= 1:
                    fillers.extend(block_fillers(c))
                if c == 2:
                    fillers.extend(
                        [lambda tt=tt: y_chunk(tt) for tt in range(0, 4)])
                if c == 3:
                    fillers.extend(
                        [lambda tt=tt: y_chunk(tt) for tt in range(4, 12)])
                for pp in range(2):
                    for g in range(2):
                        sweep(pp, c, g, FILL_EVERY[c])
                # anything not yet emitted (proj must precede block c+1)
                while fillers:
                    pop_filler()

            drain(0)
            # tail: each chunk on its own psum banks, no bank-reuse waits
            for i, tt in enumerate(range(12, 16)):
                y_chunk(tt, tail=True,
                        pt_t=[S2[0], S2[1], wk_t, po][i])

    nc.compile()
    return nc


def _prep(inputs):
    a = {k: np.asarray(v) for k, v in inputs.items()}
    lam = float(
        np.exp(np.sum(a["lq1"] * a["lk1"], dtype=np.float32))
        - np.exp(np.sum(a["lq2"] * a["lk2"], dtype=np.float32))
        + LAMBDA_INIT
    )
    wq_t = (a["Wq"].T / math.sqrt(HALF)).astype(np.float16)
    wk_t = a["Wk"].T.astype(np.float16)
    wv_t = a["Wv"].T.astype(np.float16)
    wo_g = (a["Wo"] * (np.tile(a["g"], H) * (1.0 - LAMBDA_INIT))[None, :]).T.astype(
        np.float16
    )
    r = np.arange(128)
    # ramp mask pair: (ma^T mb)[n, t] = -64*(n - t) for n > t else 0
    ma = (8.0 * (r[:, None] < r[None, :])).astype(np.float16)       # [d, n]
    mb = (-8.0 * (r[:, None] >= r[None, :])).astype(np.float16)     # [d, t]

    in_maps = []
    for core in range(N_CORES):
        b, hs = core // 4, 4 * (core % 4)
        sl = slice(DH * hs, DH * hs + DH * HPC)
        in_maps.append({
            "xq": np.ascontiguousarray(a["query"][b].T).astype(np.float16),
            "xk": np.ascontiguousarray(a["key_"][b].T).astype(np.float16),
            "xv": np.ascontiguousarray(a["value"][b].T).astype(np.float16),
            "wq": np.ascontiguousarray(wq_t[:, sl]),
            "wk": np.ascontiguousarray(wk_t[:, sl]),
            "wv": np.ascontiguousarray(wv_t[:, sl]),
            "wo": np.ascontiguousarray(wo_g[sl, :]),
            "ma": ma,
            "mb": mb,
        })
    return lam, in_maps


def run(inputs, trace=False, reps=1):
    lam, in_maps = _prep(inputs)
    key = (round(lam, 6), reps)
    if key not in _CACHE:
        _CACHE[key] = _build(lam, reps)
    nc = _CACHE[key]
    res = run_bass_kernel_spmd(
        nc, in_maps, core_ids=list(range(N_CORES)), trace=trace
    )
    out = np.empty((B, T, HID), np.float32)
    for b in range(B):
        out[b] = sum(res.results[4 * b + i]["y"].astype(np.float32) for i in range(4))
    return out, res


def kernel(**inputs) -> np.ndarray:
    out, _ = run(inputs)
    return out
